# revision 1
# baseline (speedup 1.0000x reference)
"""Multi-head self-attention (B=2, N=4096, C=512, H=8) on 8 trn2 NeuronCores.

Sharding: one head per core (tensor parallel over heads). Each core:
  - computes Q^T,K^T (d-on-partitions, two batches packed on partition halves)
    and V (m-on-partitions) for its head from the full token stream,
  - runs flash-style attention per batch: S^T = K Q^T via row-tiled matmul
    pairs, exp on ScalarE (PSUM->SBUF bf16) with the 1/sqrt(Dh) scale folded
    in, P^T @ V accumulated in PSUM with an appended ones-column producing the
    softmax denominators for free,
  - projects the (unnormalized) head output through the head's w_proj slice
    and normalizes by the softmax denominator per token (per-partition
    tensor_scalar), writing a full [8192, 512] partial.
Host sums the 8 partials and adds b_proj.
"""

import numpy as np
import ml_dtypes

import concourse.bacc as bacc
import concourse.bass as bass
import concourse.mybir as mybir
import concourse.tile as tile
from concourse.bass_utils import run_bass_kernel_spmd

BF16 = ml_dtypes.bfloat16

B = 2
N = 4096          # sequence length per batch
C = 512           # channels
H = 8             # heads
DH = C // H       # 64 head dim
T = B * N         # total tokens
NB = 512          # query-block size
MC = 128          # key-chunk size
SCALE = float(DH) ** -0.5


def _attention_body(nc, tc, xt, wq, wk, wv, wp, out, n_seq, loop_k=1,
                    act_split=False):
    """Emit the per-core kernel. n_seq = per-batch sequence length.

    loop_k > 1 wraps the whole computation in a device-side loop (the body is
    idempotent) so marginal per-iteration wall time can be measured."""
    dt = mybir.dt
    cch = C // 128             # contraction chunks over C
    nblk = n_seq // NB         # query blocks per batch
    nmc = n_seq // MC          # key chunks per batch
    ntc = n_seq // 512         # 512-token chunks per batch (qkv prep)
    tpb = NB // 128            # 128-token proj chunks per query block
    EXP = mybir.ActivationFunctionType.Exp

    const = tc.alloc_tile_pool(name="const", bufs=1)
    persist = tc.alloc_tile_pool(name="persist", bufs=1)

    # constants
    wq_sb = const.tile([128, cch, DH], dt.bfloat16)
    wk_sb = const.tile([128, cch, DH], dt.bfloat16)
    wv_sb = const.tile([128, cch, DH], dt.bfloat16)
    nc.sync.dma_start(wq_sb[:], wq.rearrange("(c p) d -> p c d", p=128))
    nc.sync.dma_start(wk_sb[:], wk.rearrange("(c p) d -> p c d", p=128))
    nc.sync.dma_start(wv_sb[:], wv.rearrange("(c p) d -> p c d", p=128))
    wp_sb = const.tile([DH, C], dt.bfloat16)
    nc.sync.dma_start(wp_sb[:], wp)
    zbias = const.tile([128, 1], dt.float32)
    nc.vector.memset(zbias[:], 0.0)

    # persistent per-head tensors
    qt2 = persist.tile([128, n_seq], dt.bfloat16)   # rows 0:64 batch0 Q^T, 64:128 batch1
    kt2 = persist.tile([128, n_seq], dt.bfloat16)
    vext = [persist.tile([128, nmc * (DH + 1)], dt.bfloat16, name=f"vext{j}")
            for j in range(2)]
    otsb = [persist.tile([DH, n_seq], dt.bfloat16, name=f"otsb{j}")
            for j in range(2)]
    lsT = persist.tile([128, 2, n_seq // 128], dt.float32)
    rsb = persist.tile([128, 2, n_seq // 128], dt.float32)
    for j in range(2):
        ones_ap = vext[j][:].rearrange("p (t c) -> p t c", c=DH + 1)[:, :, DH:DH + 1]
        nc.vector.memset(ones_ap, 1.0)

    # ---------------- phase 1: Q^T/K^T/V prep ----------------
    import contextlib
    loop_cm = tc.For_i(0, loop_k, 1) if loop_k > 1 else contextlib.nullcontext()
    env = locals()
    with loop_cm:
        _phases(nc, tc, env)
    persist.release()
    const.release()


def _phases(nc, tc, env):
    """Phases 1+2, split out so they can sit inside a timing loop."""
    dt = mybir.dt
    xt = env["xt"]; wq = env["wq"]; wk = env["wk"]; wv = env["wv"]
    wp = env["wp"]; out = env["out"]; n_seq = env["n_seq"]
    cch = env["cch"]; nblk = env["nblk"]; nmc = env["nmc"]; ntc = env["ntc"]
    tpb = env["tpb"]; EXP = env["EXP"]
    wq_sb = env["wq_sb"]; wk_sb = env["wk_sb"]; wv_sb = env["wv_sb"]
    wp_sb = env["wp_sb"]; zbias = env["zbias"]
    qt2 = env["qt2"]; kt2 = env["kt2"]; vext = env["vext"]; otsb = env["otsb"]
    lsT = env["lsT"]; rsb = env["rsb"]

    with tc.tile_pool(name="xa", bufs=3) as xpool, \
         tc.tile_pool(name="prep_ps", bufs=2, space="PSUM") as pps:
        for c in range(ntc):
            xa = xpool.tile([128, cch, 512], dt.bfloat16, tag="x")
            xb = xpool.tile([128, cch, 512], dt.bfloat16, tag="x")
            nc.sync.dma_start(
                xa[:], xt[:, c * 512:(c + 1) * 512].rearrange("(k p) i -> p k i", p=128))
            nc.sync.dma_start(
                xb[:], xt[:, n_seq + c * 512:n_seq + (c + 1) * 512].rearrange(
                    "(k p) i -> p k i", p=128))
            # Q^T / K^T: [64, 512] per batch, packed on partition halves of dst
            for wsb, dst in ((wq_sb, qt2), (wk_sb, kt2)):
                psa = pps.tile([128, 512], dt.float32, tag="qka")
                psb = pps.tile([128, 512], dt.float32, tag="qkb")
                for k in range(cch):
                    nc.tensor.matmul(psa[0:DH, :], wsb[:, k, :], xa[:, k, :],
                                     start=(k == 0), stop=(k == cch - 1),
                                     tile_position=(0, 0))
                    nc.tensor.matmul(psb[DH:2 * DH, :], wsb[:, k, :], xb[:, k, :],
                                     start=(k == 0), stop=(k == cch - 1),
                                     tile_position=(0, 64))
                nc.vector.tensor_copy(dst[0:DH, c * 512:(c + 1) * 512],
                                      psa[0:DH, :])
                nc.vector.tensor_copy(dst[DH:2 * DH, c * 512:(c + 1) * 512],
                                      psb[DH:2 * DH, :])
            # V: [m, d] tiles, one per 128 tokens
            for half, xab in ((0, xa), (1, xb)):
                for mt in range(4):
                    psv = pps.tile([128, DH], dt.float32, tag="v")
                    for k in range(cch):
                        nc.tensor.matmul(psv[:], xab[:, k, mt * 128:(mt + 1) * 128],
                                         wv_sb[:, k, :],
                                         start=(k == 0), stop=(k == cch - 1))
                    ti = c * 4 + mt
                    nc.vector.tensor_copy(
                        vext[half][:, ti * (DH + 1):ti * (DH + 1) + DH], psv[:])

    # ---------------- phase 2: attention + projection ----------------
    with tc.tile_pool(name="s_ps", bufs=2, space="PSUM") as sps, \
         tc.tile_pool(name="acc_ps", bufs=1, space="PSUM") as aps, \
         tc.tile_pool(name="proj_ps", bufs=2, space="PSUM") as jps, \
         tc.tile_pool(name="ptp", bufs=3) as ptp, \
         tc.tile_pool(name="ldram", bufs=2, space="DRAM") as ldp, \
         tc.tile_pool(name="eps", bufs=2) as eps:
        def emit_proj(nb):
            """Projection + normalize + store for query block nb."""
            for j in range(2):
                for t in range(tpb):
                    gt = nb * tpb + t
                    pp = jps.tile([128, C], dt.float32, tag="pp", name="pp")
                    nc.tensor.matmul(pp[:], otsb[j][:, gt * 128:(gt + 1) * 128],
                                     wp_sb[:], start=True, stop=True)
                    ob = eps.tile([128, C], dt.float32, tag="ob", name="ob")
                    nc.vector.tensor_scalar_mul(ob[:], pp[:], rsb[:, j, gt:gt + 1])
                    nc.sync.dma_start(
                        out[j * n_seq + gt * 128: j * n_seq + (gt + 1) * 128, :],
                        ob[:])

        for nb in range(nblk):
            acc = [aps.tile([DH + 1, NB], dt.float32, tag=f"acc{j}",
                            name=f"acc{j}") for j in range(2)]
            # AV runs one step behind QK/exp so the PE queue never blocks on
            # the current exp: PE order is QK(mc+1) then AV(mc).
            pending = None   # (pt_tile, mc)
            for mc in range(nmc):
                st = sps.tile([128, 1024], dt.float32, tag="s")
                for j in range(2):
                    # S^T[mc-block, nb-block] for batch j (row-tiled pair)
                    nc.tensor.matmul(
                        st[:, j * 512:j * 512 + NB],
                        kt2[j * DH:(j + 1) * DH, mc * 128:(mc + 1) * 128],
                        qt2[j * DH:(j + 1) * DH, nb * NB:(nb + 1) * NB],
                        start=True, stop=True,
                        tile_position=(j * 64, 0))
                pt = ptp.tile([128, 1024], dt.bfloat16, tag="pt")
                if env.get("act_split"):
                    nc.scalar.activation(pt[:, 0:512], st[:, 0:512], EXP,
                                         bias=zbias[:], scale=SCALE)
                    nc.scalar.activation(pt[:, 512:1024], st[:, 512:1024], EXP,
                                         bias=zbias[:], scale=SCALE)
                else:
                    nc.scalar.activation(pt[:], st[:], EXP, bias=zbias[:], scale=SCALE)
                if pending is not None:
                    ppt, pmc = pending
                    for j in range(2):
                        nc.tensor.matmul(
                            acc[j][:],
                            vext[j][:, pmc * (DH + 1):(pmc + 1) * (DH + 1)],
                            ppt[:, j * 512:j * 512 + NB],
                            start=(pmc == 0), stop=False)
                pending = (pt, mc)
                if mc == 4 and nb > 0:
                    emit_proj(nb - 1)   # previous block's projection, mid-stream
            ppt, pmc = pending
            for j in range(2):
                nc.tensor.matmul(
                    acc[j][:],
                    vext[j][:, pmc * (DH + 1):(pmc + 1) * (DH + 1)],
                    ppt[:, j * 512:j * 512 + NB],
                    start=False, stop=True)
            # epilogue: spill unnormalized head output + softmax denominators
            lst = eps.tile([DH + 1, 2 * NB], dt.float32, tag="ls")
            for j in range(2):
                nc.vector.tensor_copy(otsb[j][:, nb * NB:(nb + 1) * NB],
                                      acc[j][0:DH, :])
                nc.vector.tensor_copy(lst[DH:DH + 1, j * NB:(j + 1) * NB],
                                      acc[j][DH:DH + 1, :])
            # transpose denominators to [128, token-chunk] layout via a DRAM
            # bounce (engines cannot cross partitions)
            ld = ldp.tile([2, NB], dt.float32, tag="ld")
            nc.sync.dma_start(
                ld[:], lst[DH:DH + 1, :].rearrange("o (j n) -> o j n", j=2))
            for j in range(2):
                nc.sync.dma_start(
                    lsT[:, j, nb * tpb:(nb + 1) * tpb],
                    ld[j, :].rearrange("(t p) -> p t", p=128))
            nc.vector.reciprocal(rsb[:, :, nb * tpb:(nb + 1) * tpb],
                                 lsT[:, :, nb * tpb:(nb + 1) * tpb])
        emit_proj(nblk - 1)


def build_kernel(n_seq=N, loop_k=1, act_split=False):
    nc = bacc.Bacc("TRN2", target_bir_lowering=False, debug=False, num_devices=8)
    dt = mybir.dt
    t_tot = 2 * n_seq
    xt = nc.dram_tensor("xt", [C, t_tot], dt.bfloat16, kind="ExternalInput").ap()
    wq = nc.dram_tensor("wq", [C, DH], dt.bfloat16, kind="ExternalInput").ap()
    wk = nc.dram_tensor("wk", [C, DH], dt.bfloat16, kind="ExternalInput").ap()
    wv = nc.dram_tensor("wv", [C, DH], dt.bfloat16, kind="ExternalInput").ap()
    wp = nc.dram_tensor("wp", [DH, C], dt.bfloat16, kind="ExternalInput").ap()
    out = nc.dram_tensor("out", [t_tot, C], dt.float32, kind="ExternalOutput").ap()
    with tile.TileContext(nc) as tc:
        _attention_body(nc, tc, xt, wq, wk, wv, wp, out, n_seq, loop_k=loop_k,
                        act_split=act_split)
    nc.compile()
    return nc


def make_in_maps(x, w_qkv, w_proj, n_seq=N):
    """Slice the full inputs into 8 per-core input maps (head per core)."""
    t_tot = 2 * n_seq
    xt = np.ascontiguousarray(x.reshape(t_tot, C).T).astype(BF16)
    in_maps = []
    for h in range(H):
        wq = np.ascontiguousarray(w_qkv[h * DH:(h + 1) * DH, :].T).astype(BF16)
        wk = np.ascontiguousarray(w_qkv[C + h * DH:C + (h + 1) * DH, :].T).astype(BF16)
        wv = np.ascontiguousarray(
            w_qkv[2 * C + h * DH:2 * C + (h + 1) * DH, :].T).astype(BF16)
        wp = np.ascontiguousarray(w_proj[:, h * DH:(h + 1) * DH].T).astype(BF16)
        in_maps.append({"xt": xt, "wq": wq, "wk": wk, "wv": wv, "wp": wp})
    return in_maps


_NC_CACHE = {}


def _get_nc(n_seq=N):
    if n_seq not in _NC_CACHE:
        _NC_CACHE[n_seq] = build_kernel(n_seq)
    return _NC_CACHE[n_seq]


def run(x, w_qkv, w_proj, b_proj, trace=False, tmpdir=None):
    x = np.asarray(x, dtype=np.float32)
    w_qkv = np.asarray(w_qkv, dtype=np.float32)
    w_proj = np.asarray(w_proj, dtype=np.float32)
    b_proj = np.asarray(b_proj, dtype=np.float32)
    nc = _get_nc()
    in_maps = make_in_maps(x, w_qkv, w_proj)
    try:
        res = run_bass_kernel_spmd(nc, in_maps, list(range(H)), trace=trace,
                                   tmpdir=tmpdir)
    except ModuleNotFoundError:
        # no NTFF profiling hook in this environment
        res = run_bass_kernel_spmd(nc, in_maps, list(range(H)), trace=False,
                                   tmpdir=tmpdir)
    partial_sum = np.zeros((T, C), np.float64)
    for r in res.results:
        partial_sum += r["out"].astype(np.float64)
    full = (partial_sum + b_proj[None, :].astype(np.float64)).astype(np.float32)
    return full.reshape(B, N, C), res


def kernel(x, w_qkv, w_proj, b_proj):
    out, _ = run(x, w_qkv, w_proj, b_proj)
    return out



# revision 2
# speedup vs baseline: 1.2121x; 1.2121x over previous
"""Multi-head self-attention (B=2, N=4096, C=512, H=8) on 8 trn2 NeuronCores.

Sharding: one head per core (tensor parallel over heads). Each core computes
Q^T/K^T/V for its head from the full token stream, runs flash-style attention
(S^T = K Q^T row-tiled pairs, exp on ScalarE with the 1/sqrt(Dh) scale folded
in, P^T @ V accumulated in PSUM with an appended ones-column producing softmax
denominators), projects through the head's w_proj slice with per-token
normalization, and writes a full [8192, 512] fp32 partial. Host sums the 8
partials and adds b_proj.

This version pipelines the QKV prep with the first query block's attention so
the ScalarE exp stream (the throughput wall: 33.5M exps/core at 1 elem/cycle/
lane) starts within a few microseconds, keeps every PSUM tile bank-exclusive
(sub-bank sharing serializes PE writes against DVE reads via Tile's bank-aware
tracker), and spreads the projection matmuls across the mc loop so they never
head-of-line-block the PE queue.
"""

import numpy as np
import ml_dtypes

import concourse.bacc as bacc
import concourse.bass as bass
import concourse.mybir as mybir
import concourse.tile as tile
from concourse.bass_utils import run_bass_kernel_spmd

BF16 = ml_dtypes.bfloat16

B = 2
N = 4096          # sequence length per batch
C = 512           # channels
H = 8             # heads
DH = C // H       # 64 head dim
T = B * N         # total tokens
NB = 512          # query-block size
MC = 128          # key-chunk size
SCALE = float(DH) ** -0.5


def _emit(nc, tc, xt, wq, wk, wv, wp, out, n_seq):
    dt = mybir.dt
    cch = C // 128             # contraction chunks over C (4)
    nblk = n_seq // NB         # query blocks per batch (8)
    nmc = n_seq // MC          # key chunks per batch (32)
    ntc = n_seq // 512         # 512-token prep chunks per batch (8)
    mpc = 512 // MC            # key chunks per prep chunk (4)
    tpb = NB // 128            # 128-token proj chunks per query block (4)
    EXP = mybir.ActivationFunctionType.Exp

    const = tc.alloc_tile_pool(name="const", bufs=1)
    persist = tc.alloc_tile_pool(name="persist", bufs=1)

    # ---------------- constants ----------------
    wq_sb = const.tile([128, cch, DH], dt.bfloat16)
    wk_sb = const.tile([128, cch, DH], dt.bfloat16)
    wv_sb = const.tile([128, cch, DH], dt.bfloat16)
    nc.sync.dma_start(wq_sb[:], wq.rearrange("(c p) d -> p c d", p=128))
    nc.sync.dma_start(wk_sb[:], wk.rearrange("(c p) d -> p c d", p=128))
    nc.sync.dma_start(wv_sb[:], wv.rearrange("(c p) d -> p c d", p=128))
    wp_sb = const.tile([DH, C], dt.bfloat16)
    nc.sync.dma_start(wp_sb[:], wp)
    zbias = const.tile([128, 1], dt.float32)
    nc.vector.memset(zbias[:], 0.0)

    # ---------------- persistent per-head tensors ----------------
    qt2 = persist.tile([128, n_seq], dt.bfloat16)   # rows 0:64 batch0 Q^T, 64:128 batch1
    kt2 = persist.tile([128, n_seq], dt.bfloat16)
    vext = [persist.tile([128, nmc * (DH + 1)], dt.bfloat16, name=f"vext{j}")
            for j in range(2)]
    otsb = [persist.tile([DH, n_seq], dt.bfloat16, name=f"otsb{j}")
            for j in range(2)]
    lsT = persist.tile([128, 2, n_seq // 128], dt.float32)
    rsb = persist.tile([128, 2, n_seq // 128], dt.float32)
    for j in range(2):
        ones_ap = vext[j][:].rearrange("p (t c) -> p t c", c=DH + 1)[:, :, DH:DH + 1]
        nc.vector.memset(ones_ap, 1.0)

    # ---------------- PSUM pools (8 banks total) ----------------
    # sps: S^T tiles [128, 1024] fp32 = 2 banks x 2 bufs = 4 banks
    # aps: acc0/acc1 [65, 512] fp32 = 1 bank each = 2 banks
    # pps (prep, phase A) / jps (proj, phase B): 2 banks
    spool = tc.alloc_tile_pool(name="sps", bufs=2, space="PSUM")
    apool = tc.alloc_tile_pool(name="aps", bufs=1, space="PSUM")
    pps = tc.alloc_tile_pool(name="pps", bufs=1, space="PSUM")

    # ---------------- SBUF working pools ----------------
    xpool = tc.alloc_tile_pool(name="xa", bufs=6)
    ptp = tc.alloc_tile_pool(name="ptp", bufs=6)
    eps = tc.alloc_tile_pool(name="eps", bufs=2)
    ldp = tc.alloc_tile_pool(name="ldram", bufs=2, space="DRAM")

    # warm the exp table set (~2.7us ACT_TABLE_LOAD) while prep c0 runs
    warm = const.tile([128, 1], dt.float32)
    nc.scalar.activation(warm[:], zbias[:], EXP, bias=zbias[:], scale=1.0)

    # ---------------- emission helpers ----------------
    xtiles = {}

    def emit_xload(c):
        xa = xpool.tile([128, cch, 512], dt.bfloat16, tag="x", name="xa")
        xb = xpool.tile([128, cch, 512], dt.bfloat16, tag="x", name="xb")
        nc.sync.dma_start(
            xa[:], xt[:, c * 512:(c + 1) * 512].rearrange("(k p) i -> p k i", p=128))
        nc.sync.dma_start(
            xb[:], xt[:, n_seq + c * 512:n_seq + (c + 1) * 512].rearrange(
                "(k p) i -> p k i", p=128))
        xtiles[c] = (xa, xb)

    def prep_chunk(c):
        """Q^T/K^T (col-packed batch pairs) + V for 512-token chunk c."""
        xa, xb = xtiles.pop(c)
        # Q^T pair: one bank, batch halves on partition halves
        psq = pps.tile([128, 512], dt.float32, tag="qk", name="psq")
        for k in range(cch):
            nc.tensor.matmul(psq[0:DH, :], wq_sb[:, k, :], xa[:, k, :],
                             start=(k == 0), stop=(k == cch - 1),
                             tile_position=(0, 0))
            nc.tensor.matmul(psq[DH:2 * DH, :], wq_sb[:, k, :], xb[:, k, :],
                             start=(k == 0), stop=(k == cch - 1),
                             tile_position=(0, 64))
        # V batch0: 4 token-tiles into one bank
        psv0 = pps.tile([128, 256], dt.float32, tag="v", name="psv0",
                        padded_shape=(128, 512))
        for mt in range(4):
            for k in range(cch):
                nc.tensor.matmul(psv0[:, mt * DH:(mt + 1) * DH],
                                 xa[:, k, mt * 128:(mt + 1) * 128],
                                 wv_sb[:, k, :],
                                 start=(k == 0), stop=(k == cch - 1))
        nc.vector.tensor_copy(qt2[:, c * 512:(c + 1) * 512], psq[:])
        vdst0 = vext[0][:].rearrange("p (t c2) -> p t c2", c2=DH + 1)[
            :, mpc * c:mpc * (c + 1), 0:DH]
        nc.vector.tensor_copy(vdst0, psv0[:].rearrange("p (t d) -> p t d", d=DH))
        # K^T pair (qk slot reuse waits on the Q cast)
        psk = pps.tile([128, 512], dt.float32, tag="qk", name="psk")
        for k in range(cch):
            nc.tensor.matmul(psk[0:DH, :], wk_sb[:, k, :], xa[:, k, :],
                             start=(k == 0), stop=(k == cch - 1),
                             tile_position=(0, 0))
            nc.tensor.matmul(psk[DH:2 * DH, :], wk_sb[:, k, :], xb[:, k, :],
                             start=(k == 0), stop=(k == cch - 1),
                             tile_position=(0, 64))
        # V batch1
        psv1 = pps.tile([128, 256], dt.float32, tag="v", name="psv1",
                        padded_shape=(128, 512))
        for mt in range(4):
            for k in range(cch):
                nc.tensor.matmul(psv1[:, mt * DH:(mt + 1) * DH],
                                 xb[:, k, mt * 128:(mt + 1) * 128],
                                 wv_sb[:, k, :],
                                 start=(k == 0), stop=(k == cch - 1))
        nc.vector.tensor_copy(kt2[:, c * 512:(c + 1) * 512], psk[:])
        vdst1 = vext[1][:].rearrange("p (t c2) -> p t c2", c2=DH + 1)[
            :, mpc * c:mpc * (c + 1), 0:DH]
        nc.vector.tensor_copy(vdst1, psv1[:].rearrange("p (t d) -> p t d", d=DH))

    def emit_qk(nb, mc):
        """S^T tile for key-chunk mc vs query block nb, both batches; + exp."""
        st = spool.tile([128, 1024], dt.float32, tag="s", name="st")
        for j in range(2):
            nc.tensor.matmul(
                st[:, j * 512:j * 512 + NB],
                kt2[j * DH:(j + 1) * DH, mc * MC:(mc + 1) * MC],
                qt2[j * DH:(j + 1) * DH, nb * NB:(nb + 1) * NB],
                start=True, stop=True,
                tile_position=(j * DH, 0))
        pt = ptp.tile([128, 1024], dt.bfloat16, tag="pt", name="pt")
        nc.scalar.activation(pt[:], st[:], EXP, bias=zbias[:], scale=SCALE)
        return pt

    def emit_av(mc, pt, acc):
        first, last = (mc == 0), (mc == nmc - 1)
        for j in range(2):
            nc.tensor.matmul(
                acc[j][:],
                vext[j][:, mc * (DH + 1):(mc + 1) * (DH + 1)],
                pt[:, j * 512:j * 512 + NB],
                start=first, stop=last)

    def emit_epilogue(nb, acc):
        """Spill unnormalized head output + transpose softmax denominators."""
        lst = eps.tile([DH + 1, 2 * NB], dt.float32, tag="ls", name="lst")
        for j in range(2):
            nc.vector.tensor_copy(otsb[j][:, nb * NB:(nb + 1) * NB],
                                  acc[j][0:DH, :])
            nc.vector.tensor_copy(lst[DH:DH + 1, j * NB:(j + 1) * NB],
                                  acc[j][DH:DH + 1, :])
        # engines cannot cross partitions: bounce denominators через DRAM
        ld = ldp.tile([2, NB], dt.float32, tag="ld", name="ld")
        nc.sync.dma_start(
            ld[:], lst[DH:DH + 1, :].rearrange("o (j n) -> o j n", j=2))
        for j in range(2):
            nc.sync.dma_start(
                lsT[:, j, nb * tpb:(nb + 1) * tpb],
                ld[j, :].rearrange("(t p) -> p t", p=128))
        nc.vector.reciprocal(rsb[:, :, nb * tpb:(nb + 1) * tpb],
                             lsT[:, :, nb * tpb:(nb + 1) * tpb])

    def emit_proj_unit(nb, u, jpool):
        """One projection tile (j, t) = divmod(u, tpb) for query block nb."""
        j, t = divmod(u, tpb)
        gt = nb * tpb + t
        pp = jpool.tile([128, C], dt.float32, tag="pp", name="pp")
        nc.tensor.matmul(pp[:], otsb[j][:, gt * 128:(gt + 1) * 128],
                         wp_sb[:], start=True, stop=True)
        ob = eps.tile([128, C], dt.float32, tag="ob", name="ob")
        nc.vector.tensor_scalar_mul(ob[:], pp[:], rsb[:, j, gt:gt + 1])
        nc.sync.dma_start(
            out[j * n_seq + gt * 128: j * n_seq + (gt + 1) * 128, :], ob[:])

    # ---------------- phase A: prep pipelined with nb=0 attention ----------
    emit_xload(0)
    emit_xload(1)
    acc = [apool.tile([DH + 1, NB], dt.float32, tag=f"acc{j}", name=f"acc{j}")
           for j in range(2)]
    pend = []
    for c in range(ntc):
        if c + 2 < ntc:
            emit_xload(c + 2)
        prep_chunk(c)
        for mc in range(mpc * c, mpc * (c + 1)):
            pend.append((mc, emit_qk(0, mc)))
        while len(pend) > mpc:
            mc0, pt0 = pend.pop(0)
            emit_av(mc0, pt0, acc)
    for mc0, pt0 in pend:
        emit_av(mc0, pt0, acc)
    emit_epilogue(0, acc)
    pps.release()

    # ---------------- phase B: nb = 1..7, ScalarE-paced steady state --------
    jps = tc.alloc_tile_pool(name="jps", bufs=2, space="PSUM")
    for nb in range(1, nblk):
        acc = [apool.tile([DH + 1, NB], dt.float32, tag=f"acc{j}",
                          name=f"acc{j}") for j in range(2)]
        pend = []
        pu = 0
        for mc in range(nmc):
            pend.append((mc, emit_qk(nb, mc)))
            if mc >= 6 and (mc - 6) % 3 == 0 and pu < 2 * tpb:
                emit_proj_unit(nb - 1, pu, jps)   # spread: no PE queue pileup
                pu += 1
            if len(pend) > 2:
                mc0, pt0 = pend.pop(0)
                emit_av(mc0, pt0, acc)
        for mc0, pt0 in pend:
            emit_av(mc0, pt0, acc)
        emit_epilogue(nb, acc)
    for u in range(2 * tpb):
        emit_proj_unit(nblk - 1, u, jps)

    jps.release()
    ldp.release()
    eps.release()
    ptp.release()
    xpool.release()
    apool.release()
    spool.release()
    persist.release()
    const.release()


def build_kernel(n_seq=N):
    nc = bacc.Bacc("TRN2", target_bir_lowering=False, debug=False, num_devices=8)
    dt = mybir.dt
    t_tot = 2 * n_seq
    xt = nc.dram_tensor("xt", [C, t_tot], dt.bfloat16, kind="ExternalInput").ap()
    wq = nc.dram_tensor("wq", [C, DH], dt.bfloat16, kind="ExternalInput").ap()
    wk = nc.dram_tensor("wk", [C, DH], dt.bfloat16, kind="ExternalInput").ap()
    wv = nc.dram_tensor("wv", [C, DH], dt.bfloat16, kind="ExternalInput").ap()
    wp = nc.dram_tensor("wp", [DH, C], dt.bfloat16, kind="ExternalInput").ap()
    out = nc.dram_tensor("out", [t_tot, C], dt.float32, kind="ExternalOutput").ap()
    with tile.TileContext(nc) as tc:
        _emit(nc, tc, xt, wq, wk, wv, wp, out, n_seq)
    nc.compile()
    return nc


def make_in_maps(x, w_qkv, w_proj, n_seq=N):
    """Slice the full inputs into 8 per-core input maps (head per core)."""
    t_tot = 2 * n_seq
    xt = np.ascontiguousarray(x.reshape(t_tot, C).T).astype(BF16)
    in_maps = []
    for h in range(H):
        wq = np.ascontiguousarray(w_qkv[h * DH:(h + 1) * DH, :].T).astype(BF16)
        wk = np.ascontiguousarray(w_qkv[C + h * DH:C + (h + 1) * DH, :].T).astype(BF16)
        wv = np.ascontiguousarray(
            w_qkv[2 * C + h * DH:2 * C + (h + 1) * DH, :].T).astype(BF16)
        wp = np.ascontiguousarray(w_proj[:, h * DH:(h + 1) * DH].T).astype(BF16)
        in_maps.append({"xt": xt, "wq": wq, "wk": wk, "wv": wv, "wp": wp})
    return in_maps


_NC_CACHE = {}


def _get_nc(n_seq=N):
    if n_seq not in _NC_CACHE:
        _NC_CACHE[n_seq] = build_kernel(n_seq)
    return _NC_CACHE[n_seq]


def run(x, w_qkv, w_proj, b_proj, trace=False, tmpdir=None):
    x = np.asarray(x, dtype=np.float32)
    w_qkv = np.asarray(w_qkv, dtype=np.float32)
    w_proj = np.asarray(w_proj, dtype=np.float32)
    b_proj = np.asarray(b_proj, dtype=np.float32)
    nc = _get_nc()
    in_maps = make_in_maps(x, w_qkv, w_proj)
    try:
        res = run_bass_kernel_spmd(nc, in_maps, list(range(H)), trace=trace,
                                   tmpdir=tmpdir)
    except ModuleNotFoundError:
        # no NTFF profiling hook in this environment
        res = run_bass_kernel_spmd(nc, in_maps, list(range(H)), trace=False,
                                   tmpdir=tmpdir)
    partial_sum = np.zeros((T, C), np.float64)
    for r in res.results:
        partial_sum += r["out"].astype(np.float64)
    full = (partial_sum + b_proj[None, :].astype(np.float64)).astype(np.float32)
    return full.reshape(B, N, C), res


def kernel(x, w_qkv, w_proj, b_proj):
    out, _ = run(x, w_qkv, w_proj, b_proj)
    return out
